# revision 22
# baseline (speedup 1.0000x reference)
"""KNN block-edge kernel for Trainium2 (8 NeuronCores, one segment per core).

Problem (hardcoded from the reference):
  B=8 segments x NPER=512 blocks x U=4 units, 3-D positions, K=16.
  Candidate edges = all intra-segment block pairs (row-major, C=512 per row).
  Block-block distance = min over the 4x4 unit pairs of Euclidean distance.
  Output = per row the K nearest candidate edges, distance-ascending
  (ties: ascending edge index), as (row_o, col_o, attr) int32 arrays.

Device strategy per core (segment b) — "reduce and ship":
  PE computes s(iu, jv) = 2*x.y - |x|^2 - |y|^2 = -d2 for unit pairs via
  K=30 bf16 matmuls: each fp32 operand column is split into three bf16
  terms (hi/mid/lo) and six cross-term groups accumulate in fp32 PSUM,
  giving ~2^-24 relative accuracy at bf16 streaming speed.  Operands live
  on a single 30-partition band (no row-group replication) so the input
  is only 246KB, DMA'd in compute-ordered pieces on the two HW queues.
  Only the 10 upper-triangle 128x128-block chunks are computed.  The rhs
  is laid out v-major (col = c*512 + v*128 + j) so diagonal chunks can
  drop the redundant quarter of their unit-pair scores: they compute only
  S = {(u, v): (v-u) mod 4 in {0,1,2}} (12 of 16; S and its transpose
  cover all 16) as one partial tile P that the host completes with
  max(P, P.T).
  Per chunk the (u,v) min-pool runs as DVE tensor_reduce (axis XY, max
  of s) straight out of PSUM into SBUF — the measured DVE floor (1.04ns
  per element; no fast modes exist on this silicon, ACT cannot do
  two-input ops and GPSIMD cannot read PSUM, so nothing can share it).
  Pooled [128,128] fp32 tiles stream to HBM on both HW DMA queues,
  overlapped with compute.  The host assembles the symmetric score
  matrix and does the exact fp32 top-16 per row (reference tie
  semantics) with vectorized numpy.
"""

import numpy as np

B = 8
NPER = 512
U = 4
KTOP = 16
NU = NPER * U          # units per segment (2048)
NBLK = B * NPER        # total blocks (4096)
MT = NPER // 128       # row tiles per core (4)
KC = 30                # contraction: 6 groups x 5 (3-term bf16 split)
LCOLS = NU             # lhsT region columns (one per unit, t-major)
RCOLS = NU             # rhs region columns (v-major within each c-region)

# upper-triangle chunk schedule (t = row tile, c = col tile); off-diagonal
# chunks first (the first reduce only waits for 4 matmuls, and the PE gains
# slack before the shorter diagonal reduces bunched at the end)
CHUNKS = [(0, 1), (0, 2), (0, 3), (1, 2), (1, 3), (2, 3),
          (0, 0), (1, 1), (2, 2), (3, 3)]
NSLOT = len(CHUNKS)

_cache = {}


def _build_bass():
    import concourse.bacc as bacc
    import concourse.mybir as mybir
    from concourse.tile import TileContext

    f32 = mybir.dt.float32
    bf16 = mybir.dt.bfloat16
    Alu = mybir.AluOpType
    XY = mybir.AxisListType.XY

    nc = bacc.Bacc("TRN2")
    # cols [0, 2048): lhsT, col = t*512 + u*128 + i  (block t*128+i, unit u)
    # cols [2048, 4096): rhs, col = c*512 + v*128 + j  (block c*128+j, unit v)
    ops = nc.dram_tensor("ops", [KC, LCOLS + RCOLS], bf16, kind="ExternalInput")
    score = nc.dram_tensor("score", [128, NSLOT * 128], f32,
                           kind="ExternalOutput")

    with TileContext(nc) as tc:
        with (
            tc.tile_pool(name="const", bufs=1) as cpool,
            tc.tile_pool(name="psum", bufs=1, space="PSUM") as ppool,
        ):
            ops_sb = cpool.tile([KC, LCOLS + RCOLS], bf16)
            R = LCOLS  # rhs region offset

            # input pieces ordered by first use; alternate the two HW queues
            def dma_in(qi, lo, hi):
                eng = nc.sync if qi == 0 else nc.scalar
                eng.dma_start(out=ops_sb[:, lo:hi], in_=ops[:, lo:hi])

            # chunk (0,1)'s operands split in half so its first sub-matmuls
            # start while the second halves are still in flight
            dma_in(0, 0, 256)                 # lhs t0 u01
            dma_in(1, R + 512, R + 768)       # rhs c1 v01
            dma_in(0, 256, 512)               # lhs t0 u23
            dma_in(1, R + 768, R + 1024)      # rhs c1 v23
            dma_in(0, R + 1024, R + 1536)     # rhs c2
            dma_in(1, R + 1536, R + 2048)     # rhs c3
            dma_in(0, 512, 1024)              # lhs t1
            dma_in(1, 1024, 1536)             # lhs t2
            dma_in(0, R, R + 512)             # rhs c0 (diag chunks, late)
            dma_in(1, 1536, 2048)             # lhs t3

            out_sb = cpool.tile([128, NSLOT * 128], f32)

            def mm(ps, plo, t, u, rlo, rhi):
                nc.tensor.matmul(
                    ps[:, plo:plo + (rhi - rlo)],
                    lhsT=ops_sb[:, t * 512 + u * 128:t * 512 + (u + 1) * 128],
                    rhs=ops_sb[:, R + rlo:R + rhi],
                    start=True,
                    stop=True,
                )

            slot = 0
            for idx, (t, c) in enumerate(CHUNKS):
                last = idx == len(CHUNKS) - 1
                ps = ppool.tile([128, NU], f32, tag="chunk", bufs=2, name="ps")
                dst = out_sb[:, slot * 128:(slot + 1) * 128]
                eng = nc.sync if (slot % 2 == 0) else nc.scalar
                if t == c:
                    # S-set: per u the 3 v-blocks v = u, u+1, u+2 (mod 4);
                    # S + S^T covers all 16 (u,v) so the host recovers the
                    # full pool as max(P, P.T).  v-order within a u-region
                    # is irrelevant (max); wrapped ranges use 2 matmuls.
                    # Each u-region sits at a bank-aligned 512 slot (matmul
                    # output cannot cross a PSUM bank); only 384 are written
                    # and the reduce view skips the stale tail.
                    for u in range(U):
                        lo, hi = u * 128, (u + 3) * 128
                        if hi <= 512:
                            mm(ps, u * 512, t, u, c * 512 + lo, c * 512 + hi)
                        else:
                            w1 = 512 - lo
                            mm(ps, u * 512, t, u, c * 512 + lo, c * 512 + 512)
                            mm(ps, u * 512 + w1, t, u,
                               c * 512, c * 512 + hi - 512)
                    ps3 = ps.rearrange(
                        "p (u v j) -> p j u v", u=4, v=4)[:, :, :, 0:3]
                elif idx == 0:
                    # 256-col sub-matmuls keyed to the half-size input
                    # pieces (v-major keeps each v-pair contiguous and
                    # PSUM-bank-safe)
                    for u in range(U):
                        for hf in range(2):
                            mm(ps, u * 512 + hf * 256, t, u,
                               c * 512 + hf * 256, c * 512 + hf * 256 + 256)
                    ps3 = ps.rearrange("p (u v j) -> p j u v", u=4, v=4)
                else:
                    for u in range(U):
                        mm(ps, u * 512, t, u, c * 512, c * 512 + 512)
                    ps3 = ps.rearrange("p (u v j) -> p j u v", u=4, v=4)
                nc.vector.tensor_reduce(dst, ps3, XY, Alu.max)
                if last:
                    # partition-split tail DMA: 64 SBUF lines per queue in
                    # parallel (the transfer is line-count-bound)
                    nc.sync.dma_start(
                        out=score[0:64, slot * 128:(slot + 1) * 128],
                        in_=dst[0:64, :])
                    nc.scalar.dma_start(
                        out=score[64:128, slot * 128:(slot + 1) * 128],
                        in_=dst[64:128, :])
                else:
                    eng.dma_start(
                        out=score[:, slot * 128:(slot + 1) * 128], in_=dst)
                slot += 1
    nc.compile()
    return nc


def _get_nc():
    if "nc" not in _cache:
        _cache["nc"] = _build_bass()
    return _cache["nc"]


def _bf16(x):
    from ml_dtypes import bfloat16
    return x.astype(bfloat16).astype(np.float32)


def _split3(x):
    h = _bf16(x)
    m = _bf16(x - h)
    l = _bf16(x - h - m)
    return h, m, l


def _make_core_inputs(unit_pos):
    """Per-core operand tensor [30, 4096] bf16 (single-band layout).

    fp32 augmented columns: A = [2x, -|x|^2, -1] (lhs), B = [y, 1, |y|^2]
    (rhs) so A.B = -d2.  Each is split into three bf16 terms (h/m/l); six
    cross-term groups stack along K (30 rows):
      lhsT rows: [Ah; Am; Al; Ah; Am; Ah]   rhs rows: [Bh; Bh; Bh; Bm; Bm; Bl]
    -> sum = AhBh+AmBh+AlBh+AhBm+AmBm+AhBl ~= A.B to ~3e-8 relative.
    lhs cols are t-major/u-minor (col = t*512+u*128+i); rhs cols are
    v-major within each 512-column c-region (col = c*512+v*128+j).
    """
    from ml_dtypes import bfloat16

    in_maps = []
    for b in range(B):
        P = np.ascontiguousarray(unit_pos[b * NU:(b + 1) * NU]).astype(
            np.float32, copy=False)
        n = (P * P).sum(axis=1, dtype=np.float32)
        A = np.concatenate(
            [2.0 * P, -n[:, None], -np.ones((NU, 1), np.float32)], axis=1)
        Bm = np.concatenate(
            [P, np.ones((NU, 1), np.float32), n[:, None]], axis=1)
        Ah, Am, Al = _split3(A)
        Bh, Bmid, Bl = _split3(Bm)
        lhs = np.concatenate([Ah, Am, Al, Ah, Am, Ah], axis=1).T  # [30, 2048]
        rhs = np.concatenate([Bh, Bh, Bh, Bmid, Bmid, Bl], axis=1).T
        # lhs col t*512+u*128+i <- unit (t*128+i)*4+u
        lperm = (np.arange(NPER)[:, None] * U + np.arange(U)[None, :])
        lperm = lperm.reshape(MT, 128, U).transpose(0, 2, 1).reshape(-1)
        # rhs col c*512+v*128+j <- unit (c*128+j)*4+v
        rperm = (np.arange(NPER)[:, None] * U + np.arange(U)[None, :])
        rperm = rperm.reshape(MT, 128, U).transpose(0, 2, 1).reshape(-1)
        ops = np.concatenate([lhs[:, lperm], rhs[:, rperm]], axis=1)
        in_maps.append({"ops": ops.astype(bfloat16)})
    return in_maps


def _run_device(in_maps, trace=False):
    from concourse.bass_utils import run_bass_kernel_spmd

    nc = _get_nc()
    return run_bass_kernel_spmd(nc, in_maps, core_ids=list(range(B)), trace=trace)


def _ref_row_topk(P, n, i_local):
    """Reference-exact (fp32) top-16 local column indices for one row."""
    Pi = P[i_local * U:(i_local + 1) * U]                     # [4, 3]
    ni = n[i_local * U:(i_local + 1) * U]
    d2 = ni[:, None] + n[None, :] - 2.0 * (Pi @ P.T).astype(np.float32)
    dist = np.sqrt(np.maximum(d2, 0.0)).reshape(U, NPER, U).min(axis=(0, 2))
    return np.argsort(dist, kind="stable")[:KTOP]


def _postprocess(results, row, col, unit_pos):
    row_mat = row.reshape(NBLK, NPER)
    col_mat = col.reshape(NBLK, NPER)
    row_o = np.empty((NBLK, KTOP), np.int32)
    col_o = np.empty((NBLK, KTOP), np.int32)
    unit_pos = np.asarray(unit_pos, np.float32)
    ridx = np.arange(NPER)[:, None]
    NC = 24  # candidates kept per row before the exact ordering pass
    for b in range(B):
        arr = np.ascontiguousarray(results[b]["score"]).view(np.float32)
        arr = arr.reshape(128, NSLOT, 128)
        d2f = np.empty((NPER, NPER), np.float32)
        for slot, (t, c) in enumerate(CHUNKS):
            P = arr[:, slot, :]
            if t == c:
                tile = -np.maximum(P, P.T)
                d2f[t * 128:(t + 1) * 128, c * 128:(c + 1) * 128] = tile
            else:
                tile = -P
                d2f[t * 128:(t + 1) * 128, c * 128:(c + 1) * 128] = tile
                d2f[c * 128:(c + 1) * 128, t * 128:(t + 1) * 128] = tile.T
        # exact fp32 top-16, reference tie semantics (d2 asc, then col asc)
        cand = np.argpartition(d2f, NC - 1, axis=1)[:, :NC]
        dc = np.take_along_axis(d2f, cand, axis=1)
        pre = np.argsort(cand, axis=1, kind="stable")    # col-ascending
        dc = np.take_along_axis(dc, pre, axis=1)
        cand = np.take_along_axis(cand, pre, axis=1)
        ordv = np.argsort(dc, axis=1, kind="stable")[:, :KTOP]
        top_idx = np.take_along_axis(cand, ordv, axis=1)
        top_d2 = np.take_along_axis(dc, ordv, axis=1)

        # integrity: self edge first with ~zero distance, finite scores
        bad = top_idx[:, 0] != np.arange(NPER)
        bad |= np.abs(top_d2[:, 0]) > 1e-2
        bad |= ~np.isfinite(top_d2).all(axis=1)
        if bad.any():
            P = unit_pos[b * NU:(b + 1) * NU]
            n = (P * P).sum(axis=1, dtype=np.float32)
            for rloc in np.flatnonzero(bad):
                top_idx[rloc] = _ref_row_topk(P, n, rloc)
        gr = slice(b * NPER, (b + 1) * NPER)
        row_o[gr] = row_mat[gr][ridx, top_idx]
        col_o[gr] = col_mat[gr][ridx, top_idx]
    attr = np.zeros(NBLK * KTOP, np.int32)
    return row_o.reshape(-1), col_o.reshape(-1), attr


def kernel(unit_pos, row, col, unit2block, segment_ids, k):
    unit_pos = np.asarray(unit_pos, dtype=np.float32)
    row = np.asarray(row, dtype=np.int32)
    col = np.asarray(col, dtype=np.int32)
    assert int(k) == KTOP
    in_maps = _make_core_inputs(unit_pos)
    res = _run_device(in_maps, trace=False)
    return _postprocess(res.results, row, col, unit_pos)
